# revision 1
# baseline (speedup 1.0000x reference)
"""Trainium2 Bass kernel for nn_BiLSTM_73074573574724.

Reference computation:
    out_lstm  = 4-layer stacked BiLSTM over x  (final layer H=20, bidirectional
                -> 40 channels; every output element is sigmoid(o)*tanh(c),
                hence strictly inside (-1, 1))
    out_soft  = softmax(out_lstm, axis=0)       # softmax over SEQ = 2048
    out       = where(out_soft >= 0.5, 1, 0)    # int32

Key mathematical fact (holds for ALL inputs x and ALL finite LSTM weights):
the final BiLSTM layer's outputs are bounded in (-1, 1), so for any
(batch, channel) column the softmax over the 2048 sequence positions obeys

    max_t softmax_t <= e^1 / (e^1 + 2047 * e^-1) = 1 / (1 + 2047*e^-2) ~= 0.0036

which is far below the 0.5 threshold (measured on the actual reference data
the max softmax value is 5.4e-4).  Therefore the output is identically zero:
out == zeros((2048, 128, 40), int32), with a ~1000x margin to the threshold.

The memory-roofline-optimal kernel therefore just has to materialize the
42 MB int32 zero output.  We shard the batch axis over the 8 NeuronCores
(per the sharding hint); each core zero-fills one SBUF chunk and streams
its 5.24 MB output shard to DRAM with contiguous DMAs at HBM bandwidth.
The input shard is also DMA'd on-chip (full_io) which overlaps with the
output writes.
"""

import numpy as np

import concourse.bass as bass
import concourse.mybir as mybir
from concourse.bass_utils import run_bass_kernel_spmd

SEQ, BATCH, CH = 2048, 128, 40
N_CORES = 8
BL = BATCH // N_CORES                  # 16 batch elements per core
OUT_ELEMS = SEQ * BL * CH              # 1,310,720 int32 per core (5.24 MB)
P = 128                                # SBUF partitions
N_CHUNKS = 8
CHUNK_COLS = OUT_ELEMS // (P * N_CHUNKS)   # 1280 int32 per partition per chunk

X_ELEMS = SEQ * BL * 3                 # 98,304 f32 per core
X_COLS = X_ELEMS // P                  # 768


def _build_nc() -> bass.Bass:
    nc = bass.Bass()
    x_in = nc.declare_dram_parameter("x", [P, X_COLS], mybir.dt.float32,
                                     isOutput=False)
    out = nc.declare_dram_parameter("out", [N_CHUNKS, P, CHUNK_COLS],
                                    mybir.dt.int32, isOutput=True)

    with (
        nc.sbuf_tensor([P, CHUNK_COLS], mybir.dt.int32) as zt,
        nc.sbuf_tensor([P, X_COLS], mybir.dt.float32) as xbuf,
        nc.semaphore("vsem") as vsem,
        nc.semaphore("dsem") as dsem,
        nc.semaphore("xsem") as xsem,
        nc.Block() as block,
    ):

        @block.vector
        def _(vector):
            vector.memset(zt[:, :], 0).then_inc(vsem, 1)

        @block.sync
        def _(sync):
            # pull the input shard on-chip (overlaps with the output stream)
            sync.dma_start(out=xbuf[:, :], in_=x_in[:, :]).then_inc(xsem, 16)
            sync.wait_ge(vsem, 1)
            for k in range(N_CHUNKS):
                # every output chunk is the same zero tile
                sync.dma_start(out=out[k, :, :], in_=zt[:, :]).then_inc(dsem, 16)
            sync.wait_ge(dsem, 16 * N_CHUNKS)
            sync.wait_ge(xsem, 16)

    return nc


_NC_CACHE = None


def kernel(x: np.ndarray, params=None, **_unused) -> np.ndarray:
    global _NC_CACHE
    if _NC_CACHE is None:
        _NC_CACHE = _build_nc()
    nc = _NC_CACHE

    x = np.asarray(x, dtype=np.float32)
    assert x.shape == (SEQ, BATCH, 3), x.shape

    in_maps = []
    for i in range(N_CORES):
        shard = np.ascontiguousarray(x[:, i * BL:(i + 1) * BL, :])
        in_maps.append({"x": shard.reshape(P, X_COLS)})

    res = run_bass_kernel_spmd(nc, in_maps, list(range(N_CORES))).results

    parts = [np.asarray(r["out"]).reshape(SEQ, BL, CH) for r in res]
    return np.concatenate(parts, axis=1).astype(np.int32)


if __name__ == "__main__":
    x = np.random.randn(SEQ, BATCH, 3).astype(np.float32)
    out = kernel(x)
    print("kernel out:", out.shape, out.dtype, "nonzero:", int(np.count_nonzero(out)))
